# revision 15
# baseline (speedup 1.0000x reference)
"""Multi-head attention forward (B=2, S=2048, D=1024, H=16) on 8 Trainium2
NeuronCores, tensor-parallel over heads (2 heads per core).

v2 — low-precision / low-power redesign of the fp32r baseline (306 us):
  - projections + scores in bf16 (same 1 cyc/row PE rate as fp32r at >=256
    free, but ~4x fewer MACs -> no HAM clock throttle), x shipped as bf16.
  - probs + V in fp8e4m3; probs@V runs in DoubleRow perf mode (two 128-deep
    k-tiles per pass -> half the PE passes).
  - V projected directly in [token, dim] orientation (stationary = x tiles)
    so no PE transposes / staging; psum is cast straight into the fp8
    augmented-V operand (ones column provides softmax denominators).
  - bk dropped entirely (softmax is invariant to per-q-row constants:
    softmax((q+bq)@(k+bk)^T) == softmax((q+bq)@k^T) row-wise); bv folded
    into the host-side bias (sum_k p_k (v+bv) = ctx + bv since sum p = 1),
    so host adds bo + bv @ Wo.T.
  - output partials in bf16 (halves out DMA); host sums in f32.
  - out-projection interleaved into attention per q-chunk to pipeline the
    tail; PSUM = sc 3x2banks + cx 2x1bank = 8 banks total.
"""
import sys
import os

sys.path.insert(0, '/opt/trn_rl_repo')

import numpy as np
import ml_dtypes
import concourse.bass as bass
import concourse.mybir as mybir
import concourse.tile as tile
from concourse import bacc, bass_utils
import contextlib

f32 = mybir.dt.float32
f32r = mybir.dt.float32r
bf16 = mybir.dt.bfloat16
fp8 = mybir.dt.float8e4
EXP = mybir.ActivationFunctionType.Exp
DR = mybir.MatmulPerfMode.DoubleRow

B, S, D, H, HD = 2, 2048, 1024, 16, 64
T = B * S              # 4096 tokens
DC = 128               # dims per core (2 heads)
KT = 8                 # feature k-tiles (D / 128)
NCH = 8                # projection chunks of 512 tokens
NKT = 16               # k-token tiles per batch (S / 128)
NQC = 4                # q chunks of 512 per (b, h)


USE_DR = os.environ.get("K_USE_DR", "1") == "1"
USE_FP8 = os.environ.get("K_USE_FP8", "1") == "1"
K_DEBUG = os.environ.get("K_DEBUG", "0") == "1"


def _build():
    nc = bacc.Bacc("TRN2", target_bir_lowering=False, debug=False)
    pdt = fp8 if USE_FP8 else bf16
    xT_d = nc.dram_tensor("xT", [D, T], bf16, kind="ExternalInput").ap()
    wqT_d = nc.dram_tensor("wqT", [D, DC], bf16, kind="ExternalInput").ap()
    wkT_d = nc.dram_tensor("wkT", [D, DC], bf16, kind="ExternalInput").ap()
    wvT_d = nc.dram_tensor("wvT", [D, DC], bf16, kind="ExternalInput").ap()
    woT_d = nc.dram_tensor("woT", [DC, D], bf16, kind="ExternalInput").ap()
    bq_d = nc.dram_tensor("bq", [DC, 1], f32, kind="ExternalInput").ap()
    out_d = nc.dram_tensor("out", [T, D], bf16, kind="ExternalOutput").ap()

    xT_ap = xT_d.rearrange("(kt p) t -> p kt t", p=128)

    with tile.TileContext(nc) as tc:
        ctx = contextlib.ExitStack()
        cpool = ctx.enter_context(tc.tile_pool(name="cpool", bufs=1))
        xpool = ctx.enter_context(tc.tile_pool(name="xpool", bufs=2))
        ppool = ctx.enter_context(tc.tile_pool(name="ppool", bufs=6))
        npool = ctx.enter_context(tc.tile_pool(name="npool", bufs=4))
        opool = ctx.enter_context(tc.tile_pool(name="opool", bufs=3))
        sc = ctx.enter_context(tc.tile_pool(name="sc", bufs=3, space="PSUM"))
        cx = ctx.enter_context(tc.tile_pool(name="cx", bufs=2, space="PSUM"))

        # ---- constants / persistent tiles ----
        wqr = cpool.tile([128, KT, DC], bf16, tag="wqr")
        wkr = cpool.tile([128, KT, DC], bf16, tag="wkr")
        wvr = cpool.tile([128, KT, DC], bf16, tag="wvr")
        nc.gpsimd.dma_start(wqr[:], wqT_d.rearrange("(kt p) m -> p kt m", p=128))
        nc.gpsimd.dma_start(wkr[:], wkT_d.rearrange("(kt p) m -> p kt m", p=128))
        nc.gpsimd.dma_start(wvr[:], wvT_d.rearrange("(kt p) m -> p kt m", p=128))
        wor = cpool.tile([128, D], bf16, tag="wor")
        nc.gpsimd.dma_start(wor[:], woT_d[:])
        bq = cpool.tile([DC, 1], f32, tag="bq")
        nc.sync.dma_start(bq[:], bq_d[:])

        ones = cpool.tile([128, 64], f32, tag="ones")
        nc.vector.memset(ones[:], 1.0)
        onesr = cpool.tile([128, 64], f32r, tag="onesr")
        nc.vector.tensor_copy(onesr[:], ones[:])

        # aug8[p, b, h, kt, :]: fp8 augmented-V stationary per (batch, head,
        # k-token-tile).  h0: v dims at cols 0..63, ones col 64 -> ctx rows
        # 0..63, denom row 64.  h1: ones col 0, v dims at cols 64..127 ->
        # ctx rows 64..127, denom row 0 (cols 1..63 zero).  kt-adjacent
        # slabs give the DoubleRow pair stride.
        aug8 = cpool.tile([128, B, 2, NKT, 128], pdt, tag="aug8")
        nc.vector.memset(aug8[:], 0.0)
        nc.vector.memset(aug8[:, :, 0, :, 64:65], 1.0)
        nc.vector.memset(aug8[:, :, 1, :, 0:1], 1.0)

        qT = cpool.tile([128, T], bf16, tag="qT")
        kT = cpool.tile([128, T], bf16, tag="kT")
        ctxT = [cpool.tile([128, S], bf16, tag=f"ctxT{b}", name=f"ctxT{b}")
                for b in range(B)]

        # ---- phase 1: q/k projections + transposed v projection ----
        def proj_chunk(ch):
            csl = slice(ch * 512, (ch + 1) * 512)
            xTr = xpool.tile([128, KT, 512], bf16, tag="xTr")
            if ch == 0:
                # split the first chunk's load per feature tile so the first
                # projection matmuls can start as soon as f=0 lands
                for f in range(KT):
                    nc.gpsimd.dma_start(xTr[:, f], xT_ap[:, f, csl])
            else:
                nc.gpsimd.dma_start(xTr[:], xT_ap[:, :, csl])
            qk = sc.tile([128, 2, 512], f32, tag="sc", name="qk")
            for i, wr in enumerate((wqr, wkr)):
                for f in range(KT):
                    nc.tensor.matmul(qk[:, i], wr[:, f], xTr[:, f],
                                     start=(f == 0), stop=(f == KT - 1))
            nc.vector.tensor_scalar_add(qT[:, csl], qk[:, 0], bq[:])
            nc.vector.tensor_copy(kT[:, csl], qk[:, 1])
            # v in [token, dim] orientation: stationary = x tile, moving = Wv
            vp = sc.tile([128, 2, 512], f32, tag="sc", name="vp")
            for j in range(4):
                tsl = slice(j * 128, (j + 1) * 128)
                for f in range(KT):
                    nc.tensor.matmul(vp[:, j // 2, (j % 2) * 128:(j % 2) * 128 + 128],
                                     xTr[:, f, tsl], wvr[:, f],
                                     start=(f == 0), stop=(f == KT - 1))
            for j in range(4):
                tt = ch * 4 + j
                b, kt = tt // NKT, tt % NKT
                vpj = vp[:, j // 2, (j % 2) * 128:(j % 2) * 128 + 128]
                nc.vector.tensor_copy(aug8[:, b, 0, kt, 0:64], vpj[:, 0:64])
                nc.vector.tensor_copy(aug8[:, b, 1, kt, 64:128], vpj[:, 64:128])

        # ---- phase 2/3: attention + output projection, per q-chunk ----
        def attention_qc(b, qc):
            qsl = slice(b * S + qc * 512, b * S + (qc + 1) * 512)
            osl = slice(qc * 512, (qc + 1) * 512)
            for h in range(2):
                hs = slice(h * 64, (h + 1) * 64)
                ctxp = cx.tile([128, 512], f32, tag="cx", name="ctxp")
                for kp in range(NKT // 2):
                    scp = sc.tile([128, 2, 512], f32, tag="sc", name="scp")
                    probs = ppool.tile([128, 2, 512], pdt, tag="pb",
                                       name="probs")
                    for j in range(2):
                        kt = kp * 2 + j
                        ksl = slice((b * NKT + kt) * 128,
                                    (b * NKT + kt + 1) * 128)
                        nc.tensor.matmul(scp[:, j], kT[hs, ksl], qT[hs, qsl],
                                         start=True, stop=True)
                    nc.scalar.activation(probs[:], scp[:], EXP, scale=0.125)
                    if USE_DR:
                        nc.tensor.matmul(ctxp[:],
                                         aug8[:, b, h, kp * 2:kp * 2 + 2, :],
                                         probs[:], start=(kp == 0),
                                         stop=(kp == NKT // 2 - 1), perf_mode=DR)
                    else:
                        for j in range(2):
                            kt = kp * 2 + j
                            nc.tensor.matmul(ctxp[:],
                                             aug8[:, b, h, kt, :],
                                             probs[:, j], start=(kt == 0),
                                             stop=(kt == NKT - 1))
                # normalization — baseline-verbatim sequences per head
                if h == 0:
                    # denom at psum row 64; ctx rows 0..63
                    srow = npool.tile([128, 512], f32r, tag="srow")
                    nc.vector.tensor_copy(srow[64:65, :], ctxp[64:65, :])
                    bcp = cx.tile([128, 512], f32, tag="cx", name="bcp")
                    nc.tensor.matmul(bcp[0:64, :], onesr[64:65, 0:64],
                                     srow[64:65, :], start=True, stop=True)
                    bcs = npool.tile([128, 512], f32, tag="bcs")
                    nc.vector.reciprocal_approx_fast(bcs[0:64, :], bcp[0:64, :])
                    nc.vector.tensor_mul(ctxT[b][0:64, osl], ctxp[0:64, :],
                                         bcs[0:64, :])
                else:
                    # denom at psum row 0; ctx rows 64..127
                    rec = npool.tile([128, 512], f32, tag="rec")
                    nc.vector.reciprocal_approx_fast(rec[0:1, :], ctxp[0:1, :])
                    bcp = cx.tile([128, 512], f32, tag="cx", name="bcp")
                    nc.tensor.matmul(bcp[64:128, :], ones[0:1, 0:64],
                                     rec[0:1, :], start=True, stop=True)
                    cst = npool.tile([128, 512], f32, tag="cst")
                    nc.vector.tensor_copy(cst[64:128, :], ctxp[64:128, :])
                    nc.vector.tensor_mul(ctxT[b][64:128, osl], cst[64:128, :],
                                         bcp[64:128, :])
            # output projection for this q-chunk (4 token-tiles of 128)
            for j in range(4):
                tt = qc * 4 + j
                op = sc.tile([128, 2, 512], f32, tag="sc", name="op")
                ost = opool.tile([128, D], bf16, tag="ost", name="ost")
                for oc in range(2):
                    nc.tensor.matmul(op[:, oc], ctxT[b][:, tt * 128:(tt + 1) * 128],
                                     wor[:, oc * 512:(oc + 1) * 512],
                                     start=True, stop=True)
                    nc.vector.tensor_copy(ost[:, oc * 512:(oc + 1) * 512],
                                          op[:, oc])
                nc.sync.dma_start(
                    out_d[b * S + tt * 128:b * S + (tt + 1) * 128, :], ost[:])

        # Emission order = scheduler priority.  Batch-1 projections emitted
        # after attention(0) are dependency-free there, so the list scheduler
        # slots them into PE gaps while attention(0) waits on exp.
        for ch in range(NCH // 2):
            proj_chunk(ch)
        for qc in range(NQC):
            attention_qc(0, qc)
        for ch in range(NCH // 2, NCH):
            proj_chunk(ch)
        for qc in range(NQC):
            attention_qc(1, qc)
        if K_DEBUG:
            dq = nc.dram_tensor("dbg_qT", [128, T], bf16,
                                kind="ExternalOutput").ap()
            dk = nc.dram_tensor("dbg_kT", [128, T], bf16,
                                kind="ExternalOutput").ap()
            da = nc.dram_tensor("dbg_aug", [128, B * 2 * NKT * 128],
                                fp8 if USE_FP8 else bf16,
                                kind="ExternalOutput").ap()
            dc0 = nc.dram_tensor("dbg_ctxT0", [128, S], bf16,
                                 kind="ExternalOutput").ap()
            dc1 = nc.dram_tensor("dbg_ctxT1", [128, S], bf16,
                                 kind="ExternalOutput").ap()
            nc.sync.dma_start(dq[:], qT[:])
            nc.sync.dma_start(dk[:], kT[:])
            nc.sync.dma_start(da[:], aug8[:].rearrange("p a b c d -> p (a b c d)"))
            nc.sync.dma_start(dc0[:], ctxT[0][:])
            nc.sync.dma_start(dc1[:], ctxT[1][:])
        ctx.close()

    nc.compile()
    return nc


_NC = None


def _prep_in_maps(inputs, Wq, bq, Wk, Wv, Wo):
    x = np.ascontiguousarray(np.asarray(inputs, dtype=np.float32).reshape(T, D))
    xT = np.ascontiguousarray(x.T).astype(ml_dtypes.bfloat16)
    Wq = np.asarray(Wq, dtype=np.float32)
    Wk = np.asarray(Wk, dtype=np.float32)
    Wv = np.asarray(Wv, dtype=np.float32)
    Wo = np.asarray(Wo, dtype=np.float32)

    in_maps = []
    for c in range(8):
        sl = slice(c * DC, (c + 1) * DC)
        in_maps.append({
            "xT": xT,
            "wqT": np.ascontiguousarray(Wq[sl].T).astype(ml_dtypes.bfloat16),
            "wkT": np.ascontiguousarray(Wk[sl].T).astype(ml_dtypes.bfloat16),
            "wvT": np.ascontiguousarray(Wv[sl].T).astype(ml_dtypes.bfloat16),
            "woT": np.ascontiguousarray(Wo[:, sl].T).astype(ml_dtypes.bfloat16),
            "bq": np.ascontiguousarray(np.asarray(bq, np.float32)[sl][:, None]),
        })
    return in_maps


def kernel(inputs, Wq, bq, Wk, bk, Wv, bv, Wo, bo):
    global _NC
    if _NC is None:
        _NC = _build()

    in_maps = _prep_in_maps(inputs, Wq, bq, Wk, Wv, Wo)
    res = bass_utils.run_bass_kernel_spmd(_NC, in_maps, core_ids=list(range(8)))
    out = res.results[0]["out"].astype(np.float32)
    for r in res.results[1:]:
        out += r["out"].astype(np.float32)
    # bk cancels in softmax; bv contributes bv @ Wo.T to every token
    out += (np.asarray(bo, np.float32)
            + np.asarray(bv, np.float32) @ np.asarray(Wo, np.float32).T)[None, :]
    return out.reshape(B, S, D)
